# revision 1
# baseline (speedup 1.0000x reference)
"""Trainium2 Bass kernel for nn_MoCA (self-attention + momentum concept attention).

Sharding: pure data parallel — batch dim (B=8) sharded 1 batch per NeuronCore,
weights/concept pool replicated. No collectives.

Per-core algorithm for one batch (C=512, L=64, HW=4096, P=256):
  th|ph = [w_theta; w_phi]*gain @ fm          (one fused fp32r matmul, I=128)
  g     = w_g*gain @ fm, PE-transposed to gT[m, l] with an appended ones
          column so the PV matmul also produces softmax denominators.
  SA    : S^T[m, n] = ph^T th (fp32r), P^T = exp(S^T) (no max subtraction —
          scores are bounded ~|55| and fp32 exp is exact to 1e-5 there),
          attnT|denom = [gT|1]^T @ P^T accumulated in PSUM.
  The torch .view reshape (attn [HW, L] -> lat [L, HW]) is a raw memory
  reinterpret: transpose attnT blocks back to [n, l], normalize by the
  denominator (per-partition scalar), DMA to a DRAM scratch [HW, L] and
  read it back as [L, HW] (same bytes).
  sa_out = w_oT*gain*gamma_sa @ lat + fm      (residual adds split DVE/Pool)
  MoCA  : S2^T = M2 @ sa_out with M2 = phi_c^T W_theta gain precomputed on
          host (enc conv folded in); same exp/PV/normalize path with the
          256-entry concept pool; out = w_oT*gain*gamma_moca @ lat2 + sa_out.
  One global PSUM pool with two rotating tags ("big" 2-bank x3, "sm"
  1-bank x2) lets consecutive phases share banks without scope barriers.
"""
import sys

if '/opt/trn_rl_repo' not in sys.path:
    sys.path.insert(0, '/opt/trn_rl_repo')

import numpy as np

C, L, H, W, P = 512, 64, 64, 64, 256
HW = H * W
B = 8
N_CORES = 8

_STATE: dict = {}


def _truncated_store(nc, sb, out_d, sa, NCC, NB):
    import concourse.mybir as mybir
    for cc in range(NCC):
        for nb in range(NB):
            ns = slice(nb * 512, (nb + 1) * 512)
            ob = sb.tile([128, 512], mybir.dt.float32, tag="ob", name="ob", bufs=3)
            nc.vector.tensor_copy(ob[:], sa[cc][:, ns])
            nc.sync.dma_start(out_d[cc * 128:(cc + 1) * 128, ns], ob[:])


def _build_program(reps=1, num_devices=N_CORES, phases=6):
    import concourse.bass as bass
    import concourse.bacc as bacc
    import concourse.mybir as mybir
    from concourse import tile
    from concourse.masks import make_identity

    dt = mybir.dt
    AFT = mybir.ActivationFunctionType
    f32, f32r, bf16 = dt.float32, dt.float32r, dt.bfloat16

    nc = bacc.Bacc("TRN2", target_bir_lowering=False, debug=False,
                   enable_asserts=False, num_devices=num_devices)

    fm_d = nc.dram_tensor("fm", [C, HW], f32, kind="ExternalInput").ap()
    wthdup_d = nc.dram_tensor("wthdup", [C, 128], f32, kind="ExternalInput").ap()
    wphdup_d = nc.dram_tensor("wphdup", [C, 128], f32, kind="ExternalInput").ap()
    wg_d = nc.dram_tensor("wg", [C, L], f32, kind="ExternalInput").ap()
    wosa_d = nc.dram_tensor("wosa", [L, C], f32, kind="ExternalInput").ap()
    womo_d = nc.dram_tensor("womo", [L, C], f32, kind="ExternalInput").ap()
    m2t_d = nc.dram_tensor("m2t", [C, P], f32, kind="ExternalInput").ap()
    phiT_d = nc.dram_tensor("phiT", [P, L], f32, kind="ExternalInput").ap()
    out_d = nc.dram_tensor("out", [C, HW], f32, kind="ExternalOutput").ap()

    NB = HW // 512          # 8 n-blocks of 512
    NM = HW // 128          # 32 m-chunks of 128
    NCC = C // 128          # 4 channel chunks

    with tile.TileContext(nc) as tc:
      for _rep in range(reps):
        with tc.tile_pool(name="sb", bufs=1) as sb, \
             tc.tile_pool(name="dram", bufs=1, space="DRAM") as dp:

            sc1 = dp.tile([HW, L], bf16, tag="sc1", name="sc1")
            sc2 = dp.tile([HW, L], bf16, tag="sc2", name="sc2")

            # ---------------- persistent tiles ----------------
            fmr = [sb.tile([128, HW], f32r, tag=f"fmr{i}", name=f"fmr{i}") for i in range(NCC)]


            lat = sb.tile([L, HW], bf16, tag="lat", name="lat")
            lat2 = sb.tile([L, HW], bf16, tag="lat2", name="lat2")
            sa = fmr  # sa_out written in-place over fmr (residual add)
            wthr = [sb.tile([128, 128], f32r, tag=f"wthr{i}", name=f"wthr{i}") for i in range(NCC)]
            wphr = [sb.tile([128, 128], f32r, tag=f"wphr{i}", name=f"wphr{i}") for i in range(NCC)]
            wgr = [sb.tile([128, L], f32r, tag=f"wgr{i}", name=f"wgr{i}") for i in range(NCC)]
            wosab = sb.tile([L, C], bf16, tag="wosab", name="wosab")
            womob = sb.tile([L, C], bf16, tag="womob", name="womob")
            m2r = [sb.tile([128, P], f32r, tag=f"m2r{i}", name=f"m2r{i}") for i in range(NCC)]
            p2w = sb.tile([128, 2 * 65], bf16, tag="p2w", name="p2w")
            id64 = sb.tile([64, 64], f32, tag="id64", name="id64")
            id65 = sb.tile([65, 65], f32, tag="id65", name="id65")

            make_identity(nc, id64[:])
            make_identity(nc, id65[:])
            nc.vector.memset(p2w[:], 1.0)

            # ---------------- phase 1: load + casts ----------------
            ld_ctx = tc.tile_pool(name="ldpool", bufs=1)
            ldsb = ld_ctx.__enter__()
            for ci in range(NCC):
                t = ldsb.tile([128, HW], f32, tag="ld", name="ld", bufs=2)
                nc.sync.dma_start(t[:], fm_d[ci * 128:(ci + 1) * 128, :])
                nc.scalar.activation(fmr[ci][:], t[:], AFT.Copy)
            for ci in range(NCC):
                t = ldsb.tile([128, 128], f32, tag="ldw", name="ldw", bufs=2)
                nc.sync.dma_start(t[:], wthdup_d[ci * 128:(ci + 1) * 128, :])
                nc.vector.tensor_copy(wthr[ci][:], t[:])
                t3 = ldsb.tile([128, 128], f32, tag="ldw", name="ldw", bufs=2)
                nc.sync.dma_start(t3[:], wphdup_d[ci * 128:(ci + 1) * 128, :])
                nc.vector.tensor_copy(wphr[ci][:], t3[:])
                t2 = ldsb.tile([128, L], f32, tag="ldg", name="ldg", bufs=2)
                nc.sync.dma_start(t2[:], wg_d[ci * 128:(ci + 1) * 128, :])
                nc.vector.tensor_copy(wgr[ci][:], t2[:])
            t = ldsb.tile([L, C], f32, tag="ldo", name="ldo", bufs=2)
            nc.sync.dma_start(t[:], wosa_d[:])
            nc.vector.tensor_copy(wosab[:], t[:])
            t = ldsb.tile([L, C], f32, tag="ldo", name="ldo", bufs=2)
            nc.sync.dma_start(t[:], womo_d[:])
            nc.vector.tensor_copy(womob[:], t[:])
            for ci in range(NCC):
                t = ldsb.tile([128, P], f32, tag="ldp", name="ldp", bufs=2)
                nc.sync.dma_start(t[:], m2t_d[ci * 128:(ci + 1) * 128, :])
                nc.vector.tensor_copy(m2r[ci][:], t[:])
            for pc in range(2):
                t = ldsb.tile([128, L], f32, tag="ldg", name="ldg", bufs=2)
                nc.sync.dma_start(t[:], phiT_d[pc * 128:(pc + 1) * 128, :])
                nc.vector.tensor_copy(p2w[:, pc * 65:pc * 65 + 64], t[:])

            ld_ctx.__exit__(None, None, None)

            # ---------------- phase 2: th and ph and g convs, gT ----------------
            if phases < 2:
                _truncated_store(nc, sb, out_d, sa, NCC, NB)
                continue
            sa_ctx = tc.tile_pool(name="sapool", bufs=1)
            sasb = sa_ctx.__enter__()
            ps_ctx = tc.tile_pool(name="ps", bufs=1, space="PSUM")
            psum = ps_ctx.__enter__()
            if True:
                th = sasb.tile([128, HW], f32r, tag="th", name="th")
                ph = sasb.tile([128, HW], f32r, tag="ph", name="ph")
                gto = sasb.tile([128, NM * 65], bf16, tag="gto", name="gto")
                nc.vector.memset(gto[:], 1.0)
                g_sb = sasb.tile([L, HW], f32, tag="g_sb", name="g_sb")
                for nb in range(NB):
                    ns = slice(nb * 512, (nb + 1) * 512)
                    pst = psum.tile([128, 512], f32, tag="big", name="th_ps", bufs=3, padded_shape=[128, 1024])
                    for ci in range(NCC):
                        nc.tensor.matmul(pst[:], wthr[ci][:], fmr[ci][:, ns],
                                         start=(ci == 0), stop=(ci == NCC - 1))
                    nc.scalar.activation(th[:, ns], pst[:], AFT.Copy)
                    psp = psum.tile([128, 512], f32, tag="big", name="ph_ps", bufs=3, padded_shape=[128, 1024])
                    for ci in range(NCC):
                        nc.tensor.matmul(psp[:], wphr[ci][:], fmr[ci][:, ns],
                                         start=(ci == 0), stop=(ci == NCC - 1))
                    nc.scalar.activation(ph[:, ns], psp[:], AFT.Copy)
                    psg = psum.tile([L, 512], f32, tag="big", name="g_ps", bufs=3, padded_shape=[128, 1024])
                    for ci in range(NCC):
                        nc.tensor.matmul(psg[:], wgr[ci][:], fmr[ci][:, ns],
                                         start=(ci == 0), stop=(ci == NCC - 1))
                    nc.scalar.activation(g_sb[:, ns], psg[:], AFT.Copy)
                for mc in range(NM):
                    tp = psum.tile([128, 64], f32, tag="sm", name="gtp", bufs=2, padded_shape=[128, 512])
                    nc.tensor.transpose(tp[:], g_sb[:, mc * 128:(mc + 1) * 128],
                                        id64[:])
                    nc.vector.tensor_copy(gto[:, mc * 65:mc * 65 + 64], tp[:])

            if phases < 3:
                sa_ctx.__exit__(None, None, None)
                _truncated_store(nc, sb, out_d, sa, NCC, NB)
                continue
            # ---------------- phase 3: self-attention ----------------
            # software-pipelined: PV lags (ST, exp) by LAG pair-steps so the
            # PE queue never stalls waiting on ACT.
            NPAIR = NM // 2          # 16 pair-steps of 2 m-chunks
            LAG = 2
            if True:
                for nb in range(NB):
                    ns = slice(nb * 512, (nb + 1) * 512)
                    pv = psum.tile([65, 512], f32, tag="sm", name="pv", bufs=2, padded_shape=[128, 512])
                    pts = {}
                    for j in range(NPAIR + LAG):
                        if j < NPAIR:
                            st = psum.tile([128, 1024], f32, tag="big", name="st", bufs=3)
                            for h in range(2):
                                mc = 2 * j + h
                                hp = slice(64 * h, 64 * h + 64)
                                nc.tensor.matmul(
                                    st[:, h * 512:(h + 1) * 512],
                                    ph[hp, mc * 128:(mc + 1) * 128],
                                    th[hp, ns], start=True, stop=True,
                                    tile_position=(64 * h, 0))
                            ptt = sasb.tile([128, 1024], bf16, tag="pt",
                                            name="pt", bufs=LAG + 4)
                            nc.scalar.activation(ptt[:], st[:], AFT.Exp)
                            pts[j] = ptt
                        if j >= LAG:
                            jj = j - LAG
                            ptt = pts.pop(jj)
                            for h in range(2):
                                mc = 2 * jj + h
                                nc.tensor.matmul(
                                    pv[:], gto[:, mc * 65:(mc + 1) * 65],
                                    ptt[:, h * 512:(h + 1) * 512],
                                    start=(mc == 0), stop=(mc == NM - 1))
                    at = sasb.tile([65, 512], f32, tag="at", name="at", bufs=2)
                    nc.vector.tensor_copy(at[:], pv[:])
                    for k in range(4):
                        tp = psum.tile([128, 65], f32, tag="sm", name="tt", bufs=2, padded_shape=[128, 512])
                        nc.tensor.transpose(tp[:], at[:, k * 128:(k + 1) * 128],
                                            id65[:])
                        rc = sasb.tile([128, 1], f32, tag="rc", name="rc", bufs=2)
                        nc.vector.reciprocal(rc[:], tp[:, 64:65])
                        tb = sasb.tile([128, 64], bf16, tag="tb", name="tb", bufs=4)
                        nc.vector.tensor_scalar_mul(tb[:], tp[:, 0:64], rc[:])
                        n0 = nb * 512 + k * 128
                        nc.sync.dma_start(sc1[n0:n0 + 128, :], tb[:])
                    lat_view = sc1[:].rearrange("(a b) c -> a (b c)", a=L)
                    nc.sync.dma_start(lat[nb * 8:(nb + 1) * 8, :],
                                      lat_view[nb * 8:(nb + 1) * 8, :])
            sa_ctx.__exit__(None, None, None)

            if phases < 4:
                _truncated_store(nc, sb, out_d, sa, NCC, NB)
                continue
            # ------- phases 4+5 merged: per-nb oconv+residual, enc, concept attn -------
            if True:
                moca = {}
                for nb in range(NB + 1):
                    if nb < NB:
                        ns = slice(nb * 512, (nb + 1) * 512)
                        for cc in range(NCC):
                            ps = psum.tile([128, 512], f32, tag="big", name="oc", bufs=3, padded_shape=[128, 1024])
                            nc.tensor.matmul(ps[:],
                                             wosab[:, cc * 128:(cc + 1) * 128],
                                             lat[:, ns], start=True, stop=True)
                            if cc < 2:
                                nc.vector.tensor_add(sa[cc][:, ns], ps[:],
                                                     sa[cc][:, ns])
                            else:
                                tmp = sb.tile([128, 512], f32, tag="rtmp",
                                              name="rtmp", bufs=3)
                                nc.scalar.activation(tmp[:], ps[:], AFT.Copy)
                                nc.gpsimd.tensor_add(sa[cc][:, ns], tmp[:],
                                                     sa[cc][:, ns])
                        s2 = psum.tile([128, 1024], f32, tag="big", name="s2", bufs=3)
                        for pc in range(2):
                            for ci in range(NCC):
                                nc.tensor.matmul(
                                    s2[:, pc * 512:(pc + 1) * 512],
                                    m2r[ci][:, pc * 128:(pc + 1) * 128],
                                    sa[ci][:, ns],
                                    start=(ci == 0), stop=(ci == NCC - 1))
                        p2t = sb.tile([128, 1024], bf16, tag="p2t", name="p2t",
                                      bufs=3)
                        nc.scalar.activation(p2t[:], s2[:], AFT.Exp)
                        moca[nb] = p2t
                    if nb >= 1:
                        nbl = nb - 1
                        p2t = moca.pop(nbl)
                        pv2 = psum.tile([65, 512], f32, tag="sm", name="pv2", bufs=2, padded_shape=[128, 512])
                        for pc in range(2):
                            nc.tensor.matmul(pv2[:], p2w[:, pc * 65:(pc + 1) * 65],
                                             p2t[:, pc * 512:(pc + 1) * 512],
                                             start=(pc == 0), stop=(pc == 1))
                        at2 = sb.tile([65, 512], f32, tag="at2", name="at2",
                                      bufs=2)
                        nc.scalar.activation(at2[:], pv2[:], AFT.Copy)
                        for k in range(4):
                            tp = psum.tile([128, 65], f32, tag="sm", name="tt2", bufs=2, padded_shape=[128, 512])
                            nc.tensor.transpose(tp[:],
                                                at2[:, k * 128:(k + 1) * 128],
                                                id65[:])
                            rc = sb.tile([128, 1], f32, tag="rc2", name="rc2",
                                         bufs=2)
                            nc.vector.reciprocal(rc[:], tp[:, 64:65])
                            tb = sb.tile([128, 64], bf16, tag="tb2", name="tb2",
                                         bufs=3)
                            nc.scalar.activation(tb[:], tp[:, 0:64], AFT.Copy,
                                                 scale=rc[:])
                            n0 = nbl * 512 + k * 128
                            nc.sync.dma_start(sc2[n0:n0 + 128, :], tb[:])
                        lat2_view = sc2[:].rearrange("(a b) c -> a (b c)", a=L)
                        nc.sync.dma_start(lat2[nbl * 8:(nbl + 1) * 8, :],
                                          lat2_view[nbl * 8:(nbl + 1) * 8, :])

            # ---------------- phase 6: final o-conv + residual ----------------
            if True:
                for nb in range(NB):
                    for cc in range(NCC):
                        ns = slice(nb * 512, (nb + 1) * 512)
                        ps = psum.tile([128, 512], f32, tag="big", name="o2", bufs=3, padded_shape=[128, 1024])
                        nc.tensor.matmul(ps[:], womob[:, cc * 128:(cc + 1) * 128],
                                         lat2[:, ns], start=True, stop=True)
                        ob = sb.tile([128, 512], f32, tag="ob", name="ob", bufs=3)
                        if cc < 2:
                            nc.vector.tensor_add(ob[:], ps[:], sa[cc][:, ns])
                        else:
                            tmp = sb.tile([128, 512], f32, tag="rtmp2",
                                          name="rtmp2", bufs=3)
                            nc.scalar.activation(tmp[:], ps[:], AFT.Copy)
                            nc.gpsimd.tensor_add(ob[:], tmp[:], sa[cc][:, ns])
                        nc.sync.dma_start(out_d[cc * 128:(cc + 1) * 128, ns], ob[:])
            ps_ctx.__exit__(None, None, None)

    nc.compile()
    return nc


def _get_runner(reps=1):
    """Build the Bass program once and return a cached jitted SPMD callable."""
    key = ("runner", reps)
    if key in _STATE:
        return _STATE[key]

    import jax
    import numpy as np
    from jax.experimental.shard_map import shard_map
    from jax.sharding import Mesh, PartitionSpec
    import concourse.mybir as mybir
    from concourse import bass2jax

    nc = _build_program(reps=reps)
    bass2jax.install_neuronx_cc_hook()

    partition_name = (nc.partition_id_tensor.name
                      if nc.partition_id_tensor else None)
    in_names, out_names, out_avals, zero_shapes = [], [], [], []
    for alloc in nc.m.functions[0].allocations:
        if not isinstance(alloc, mybir.MemoryLocationSet):
            continue
        name = alloc.memorylocations[0].name
        if alloc.kind == "ExternalInput":
            if name != partition_name:
                in_names.append(name)
        elif alloc.kind == "ExternalOutput":
            out_names.append(name)
            shape = tuple(alloc.tensor_shape)
            dtype = mybir.dt.np(alloc.dtype)
            out_avals.append(jax.core.ShapedArray(shape, dtype))
            zero_shapes.append((shape, dtype))
    n_params = len(in_names)
    all_in_names = list(in_names) + list(out_names)
    if partition_name is not None:
        all_in_names.append(partition_name)

    def _body(*args):
        operands = list(args)
        if partition_name is not None:
            operands.append(bass2jax.partition_id_tensor())
        outs = bass2jax._bass_exec_p.bind(
            *operands,
            out_avals=tuple(out_avals),
            in_names=tuple(all_in_names),
            out_names=tuple(out_names),
            lowering_input_output_aliases=(),
            sim_require_finite=True,
            sim_require_nnan=True,
            nc=nc,
        )
        return tuple(outs)

    devices = jax.devices()[:N_CORES]
    mesh = Mesh(np.asarray(devices), ("core",))
    n_outs = len(out_names)
    donate = tuple(range(n_params, n_params + n_outs))
    sharded = jax.jit(
        shard_map(_body, mesh=mesh,
                  in_specs=(PartitionSpec("core"),) * (n_params + n_outs),
                  out_specs=(PartitionSpec("core"),) * n_outs,
                  check_rep=False),
        donate_argnums=donate, keep_unused=True)

    runner = {
        "nc": nc, "sharded": sharded, "in_names": in_names,
        "out_names": out_names, "zero_shapes": zero_shapes,
        "n_params": n_params,
    }
    _STATE[key] = runner
    return runner


def _prep_in_maps(feature_map, concepts, w_theta, w_phi, w_g, w_o,
                  gamma_sa, gamma_moca):
    feature_map = np.asarray(feature_map, dtype=np.float32)
    concepts = np.asarray(concepts, dtype=np.float32)
    w_theta = np.asarray(w_theta, dtype=np.float32)
    w_phi = np.asarray(w_phi, dtype=np.float32)
    w_g = np.asarray(w_g, dtype=np.float32)
    w_o = np.asarray(w_o, dtype=np.float32)
    gamma_sa = np.float32(gamma_sa)
    gamma_moca = np.float32(gamma_moca)

    gain = np.float32(1.0 / np.sqrt(C))
    gain_o = np.float32(1.0 / np.sqrt(L))

    wth_t = w_theta.T * gain                                        # [C, L]
    wph_t = w_phi.T * gain
    wthdup = np.ascontiguousarray(np.concatenate([wth_t, wth_t], axis=1))
    wphdup = np.ascontiguousarray(np.concatenate([wph_t, wph_t], axis=1))
    wg_t = np.ascontiguousarray(w_g.T * gain)                       # [C, L]
    wosa = np.ascontiguousarray(w_o.T * (gain_o * gamma_sa))        # [L, C]
    womo = np.ascontiguousarray(w_o.T * (gain_o * gamma_moca))      # [L, C]
    m2 = concepts @ (w_theta * gain)            # [P, C]
    m2t = np.ascontiguousarray(m2.T)            # [C, P]
    phiT = np.ascontiguousarray(concepts)                           # [P, L]
    fm_flat = feature_map.reshape(B, C, HW)

    in_maps = []
    for b in range(N_CORES):
        in_maps.append({
            "fm": np.ascontiguousarray(fm_flat[b]),
            "wthdup": wthdup, "wphdup": wphdup, "wg": wg_t,
            "wosa": wosa, "womo": womo, "m2t": m2t, "phiT": phiT,
        })
    return in_maps


def _run(in_maps):
    r = _get_runner()
    n_params = r["n_params"]
    concat_in = [
        np.concatenate([np.asarray(in_maps[c][name])
                        for c in range(N_CORES)], axis=0)
        for name in r["in_names"]
    ]
    concat_zeros = [np.zeros((N_CORES * s[0], *s[1:]), d)
                    for (s, d) in r["zero_shapes"]]
    out_arrs = r["sharded"](*concat_in, *concat_zeros)
    per_core = []
    for c in range(N_CORES):
        per_core.append({
            name: np.asarray(out_arrs[i]).reshape(
                N_CORES, *r["zero_shapes"][i][0])[c]
            for i, name in enumerate(r["out_names"])
        })
    return per_core


def kernel(feature_map, concepts, w_theta, w_phi, w_g, w_o,
           gamma_sa, gamma_moca):
    in_maps = _prep_in_maps(feature_map, concepts, w_theta, w_phi, w_g, w_o,
                            gamma_sa, gamma_moca)
    per_core = _run(in_maps)
    out = np.stack([per_core[b]["out"].reshape(C, H, W)
                    for b in range(B)], axis=0)
    return out.astype(np.float32)



# revision 14
# speedup vs baseline: 16.9956x; 16.9956x over previous
"""Trainium2 Bass kernel for nn_MoCA (self-attention + momentum concept attention).

Sharding: pure data parallel — batch dim (B=8) sharded 1 batch per NeuronCore,
weights/concept pool replicated. No collectives.

v2 — restructured around the ACT-engine exp roofline (~147 us of exp per core):
  * fm loaded as 32 per-(ci,nb) chunk tiles; convs pipeline behind the DMAs
    (per-chunk DVE f32->f32r rounding copies, off the ACT engine).
  * SA phase: ACT runs ONLY exps; all PSUM drains (th/ph/g/at/s2f) are DVE.
    ST/PV software pipeline (LAG=2) with quadrant-packed K=64 score matmuls.
  * s2_fm = M2 @ fm (the enc-conv/concept matmul folded with the fm part of
    sa_out) is computed inside the SA window using PE slack, into s2f SBUF.
  * sa_out is never materialized: s2 = I@s2f + (gamma_sa*M2@w_o)@lat, and the
    final output is one merged conv out = [wosa|womo] @ [lat;lat2] + fm.
  * normalize path: PV emits [attnT|denom]; PE-transpose back to [n, l],
    multiply by reciprocal denominator, one 3D-AP DMA to a DRAM scratch
    [HW, L] and re-read as [L, HW] (the torch .view memory reinterpret).
  * PSUM: tags big(2 banks x2) / pv(1 x2) / tt(1 x2) = 8 banks.
"""
import sys

if '/opt/trn_rl_repo' not in sys.path:
    sys.path.insert(0, '/opt/trn_rl_repo')

import numpy as np

C, L, H, W, P = 512, 64, 64, 64, 256
HW = H * W
B = 8
N_CORES = 8

_STATE: dict = {}


def _build_program(reps=1, num_devices=N_CORES):
    import concourse.bass as bass
    import concourse.bacc as bacc
    import concourse.mybir as mybir
    from concourse import tile
    from concourse.masks import make_identity

    dt = mybir.dt
    AFT = mybir.ActivationFunctionType
    f32, f32r, bf16 = dt.float32, dt.float32r, dt.bfloat16

    nc = bacc.Bacc("TRN2", target_bir_lowering=False, debug=False,
                   enable_asserts=False, num_devices=num_devices)

    fm_d = nc.dram_tensor("fm", [C, HW], f32, kind="ExternalInput").ap()
    wthdup_d = nc.dram_tensor("wthdup", [C, 128], f32, kind="ExternalInput").ap()
    wphdup_d = nc.dram_tensor("wphdup", [C, 128], f32, kind="ExternalInput").ap()
    wg_d = nc.dram_tensor("wg", [C, L], f32, kind="ExternalInput").ap()
    m2t_d = nc.dram_tensor("m2t", [C, P], f32, kind="ExternalInput").ap()
    m2bt_d = nc.dram_tensor("m2bt", [L, P], f32, kind="ExternalInput").ap()
    wcat_d = nc.dram_tensor("wcat", [128, C], f32, kind="ExternalInput").ap()
    phiT_d = nc.dram_tensor("phiT", [P, L], f32, kind="ExternalInput").ap()
    out_d = nc.dram_tensor("out", [C, HW], f32, kind="ExternalOutput").ap()

    NB = HW // 512          # 8 n-blocks of 512
    NM = HW // 128          # 32 m-chunks of 128
    NCC = C // 128          # 4 channel chunks
    NPAIR = NM // 2         # 16 pair-steps per n-block
    LAG = 2

    with tile.TileContext(nc) as tc:
      for _rep in range(reps):
        with tc.tile_pool(name="sb", bufs=1) as sb, \
             tc.tile_pool(name="dram", bufs=1, space="DRAM") as dp, \
             tc.tile_pool(name="ps", bufs=1, space="PSUM") as psum:

            sc1 = dp.tile([HW, L], bf16, tag="sc1", name="sc1")
            sc2 = dp.tile([HW, L], bf16, tag="sc2", name="sc2")

            # ---------------- persistent tiles ----------------
            fmc = [[sb.tile([128, 512], f32r, tag=f"fmc{ci}_{nb}",
                            name=f"fmc{ci}_{nb}")
                    for nb in range(NB)] for ci in range(NCC)]
            th = sb.tile([128, HW], f32r, tag="th", name="th")
            ph = sb.tile([128, HW], f32r, tag="ph", name="ph")
            g_sb = sb.tile([L, HW], bf16, tag="g_sb", name="g_sb")
            gto = sb.tile([128, NM * 65], bf16, tag="gto", name="gto")
            s2f = [sb.tile([128, HW], f32r, tag=f"s2f{i}", name=f"s2f{i}")
                   for i in range(2)]
            latcat = sb.tile([128, HW], bf16, tag="latcat", name="latcat")

            wthr = [sb.tile([128, 128], f32r, tag=f"wthr{i}", name=f"wthr{i}")
                    for i in range(NCC)]
            wphr = [sb.tile([128, 128], f32r, tag=f"wphr{i}", name=f"wphr{i}")
                    for i in range(NCC)]
            wgr = [sb.tile([128, L], f32r, tag=f"wgr{i}", name=f"wgr{i}")
                   for i in range(NCC)]
            m2r = [sb.tile([128, P], f32r, tag=f"m2r{i}", name=f"m2r{i}")
                   for i in range(NCC)]
            m2bb = sb.tile([L, P], bf16, tag="m2bb", name="m2bb")
            wcatb = sb.tile([128, C], bf16, tag="wcatb", name="wcatb")
            p2w = sb.tile([128, 2 * 65], bf16, tag="p2w", name="p2w")
            id64 = sb.tile([64, 64], bf16, tag="id64", name="id64")
            id65 = sb.tile([65, 65], f32, tag="id65", name="id65")
            id128 = sb.tile([128, 128], f32, tag="id128", name="id128")
            id128r = sb.tile([128, 128], f32r, tag="id128r", name="id128r")

            make_identity(nc, id64[:])
            make_identity(nc, id65[:])
            make_identity(nc, id128[:])
            nc.vector.tensor_copy(id128r[:], id128[:])
            nc.vector.memset(p2w[:], 1.0)
            nc.vector.memset(gto[:], 1.0)

            # ---------------- weight loads (small, first) ----------------
            for ci in range(NCC):
                tw = sb.tile([128, 128], f32, tag="tw", name="tw", bufs=2)
                nc.sync.dma_start(tw[:], wthdup_d[ci * 128:(ci + 1) * 128, :])
                nc.vector.tensor_copy(wthr[ci][:], tw[:])
                tw = sb.tile([128, 128], f32, tag="tw", name="tw", bufs=2)
                nc.sync.dma_start(tw[:], wphdup_d[ci * 128:(ci + 1) * 128, :])
                nc.vector.tensor_copy(wphr[ci][:], tw[:])
                tw = sb.tile([128, L], f32, tag="twg", name="twg", bufs=2)
                nc.sync.dma_start(tw[:], wg_d[ci * 128:(ci + 1) * 128, :])
                nc.vector.tensor_copy(wgr[ci][:], tw[:])
                tw = sb.tile([128, P], f32, tag="twm", name="twm", bufs=2)
                nc.sync.dma_start(tw[:], m2t_d[ci * 128:(ci + 1) * 128, :])
                nc.vector.tensor_copy(m2r[ci][:], tw[:])
            twc = sb.tile([128, C], f32, tag="twc", name="twc")
            nc.sync.dma_start(twc[:], wcat_d[:])
            nc.vector.tensor_copy(wcatb[:], twc[:])
            tmb = sb.tile([L, P], f32, tag="tmb", name="tmb")
            nc.sync.dma_start(tmb[:], m2bt_d[:])
            nc.vector.tensor_copy(m2bb[:], tmb[:])
            for pc in range(2):
                tph = sb.tile([128, L], f32, tag="tph", name="tph", bufs=2)
                nc.sync.dma_start(tph[:], phiT_d[pc * 128:(pc + 1) * 128, :])
                nc.vector.tensor_copy(p2w[:, pc * 65:pc * 65 + 64], tph[:])

            # ---------------- fm chunk loads (nb-major) + f32r rounding ----
            for nb in range(NB):
                ns = slice(nb * 512, (nb + 1) * 512)
                for ci in range(NCC):
                    tf = sb.tile([128, 512], f32, tag="tf", name="tf", bufs=2)
                    nc.sync.dma_start(tf[:], fm_d[ci * 128:(ci + 1) * 128, ns])
                    nc.vector.tensor_copy(fmc[ci][nb][:], tf[:])

            # ---------------- head: th/ph/g convs + gT ----------------
            for nb in range(NB):
                ns = slice(nb * 512, (nb + 1) * 512)
                thph = psum.tile([128, 1024], f32, tag="big", name="thph",
                                 bufs=2)
                for ci in range(NCC):
                    nc.tensor.matmul(thph[:, 0:512],
                                     wthr[ci][:],
                                     fmc[ci][nb][:],
                                     start=(ci == 0), stop=(ci == NCC - 1))
                for ci in range(NCC):
                    nc.tensor.matmul(thph[:, 512:1024],
                                     wphr[ci][:],
                                     fmc[ci][nb][:],
                                     start=(ci == 0), stop=(ci == NCC - 1))
                nc.vector.tensor_copy(th[:, ns], thph[:, 0:512])
                nc.vector.tensor_copy(ph[:, ns], thph[:, 512:1024])

                psg = psum.tile([L, 512], f32, tag="tt", name="g_ps", bufs=2,
                                padded_shape=[128, 512])
                for ci in range(NCC):
                    nc.tensor.matmul(psg[:], wgr[ci][:],
                                     fmc[ci][nb][:],
                                     start=(ci == 0), stop=(ci == NCC - 1))
                nc.vector.tensor_copy(g_sb[:, ns], psg[:])
                for k in range(4):
                    mc = nb * 4 + k
                    tp = psum.tile([128, 64], bf16, tag="pv", name="gtp",
                                   bufs=2, padded_shape=[128, 1024])
                    nc.tensor.transpose(tp[:], g_sb[:, mc * 128:(mc + 1) * 128],
                                        id64[:])
                    nc.vector.tensor_copy(gto[:, mc * 65:mc * 65 + 64], tp[:])

            # ---------------- SA + folded M2@fm, 1-nb-lagged normalize ----
            def normalize_rest(nbl, at, tbt, scr, dst_row0):
                """PE-transpose attnT blocks, divide by denom, 1 DMA to DRAM
                scratch, 1 DMA back as [L, HW] rows."""
                for k in range(4):
                    tp = psum.tile([128, 65], f32, tag="tt", name="tt",
                                   bufs=2, padded_shape=[128, 512])
                    nc.tensor.transpose(tp[:], at[:, k * 128:(k + 1) * 128],
                                        id65[:])
                    rc = sb.tile([128, 1], f32, tag="rc", name="rc", bufs=2)
                    nc.vector.reciprocal(rc[:], tp[:, 64:65])
                    nc.vector.tensor_scalar_mul(tbt[:, k * 64:(k + 1) * 64],
                                                tp[:, 0:64], rc[:])
                scr_view = scr[nbl * 512:(nbl + 1) * 512, :].rearrange(
                    "(k p) c -> p k c", k=4)
                tbt_view = tbt[:].rearrange("p (k c) -> p k c", k=4)
                nc.sync.dma_start(scr_view, tbt_view)
                lat_view = scr[:].rearrange("(a b) c -> a (b c)", a=L)
                nc.sync.dma_start(latcat[dst_row0 + nbl * 8:
                                         dst_row0 + (nbl + 1) * 8, :],
                                  lat_view[nbl * 8:(nbl + 1) * 8, :])

            sa_norm = {}
            for nb in range(NB):
                ns = slice(nb * 512, (nb + 1) * 512)
                pv = psum.tile([65, 512], f32, tag="pv", name="pv", bufs=2,
                               padded_shape=[128, 512])
                pts = {}
                for j in range(NPAIR + LAG):
                    if j < NPAIR:
                        st = psum.tile([128, 1024], f32, tag="big", name="st",
                                       bufs=2)
                        for h in range(2):
                            mc = 2 * j + h
                            hp = slice(64 * h, 64 * h + 64)
                            nc.tensor.matmul(
                                st[:, h * 512:(h + 1) * 512],
                                ph[hp, mc * 128:(mc + 1) * 128],
                                th[hp, ns],
                                start=True, stop=True,
                                tile_position=(64 * h, 0))
                        ptt = sb.tile([128, 1024], bf16, tag="pt", name="pt",
                                      bufs=LAG + 2)
                        nc.scalar.activation(ptt[:], st[:], AFT.Exp)
                        pts[j] = ptt
                    if j == 1 and nb >= 1:
                        normalize_rest(nb - 1, *sa_norm.pop(nb - 1), sc1, 0)
                    if j >= LAG:
                        jj = j - LAG
                        ptt = pts.pop(jj)
                        for h in range(2):
                            mc = 2 * jj + h
                            nc.tensor.matmul(
                                pv[:], gto[:, mc * 65:(mc + 1) * 65],
                                ptt[:, h * 512:(h + 1) * 512],
                                start=(mc == 0), stop=(mc == NM - 1))
                # M2 @ fm slice for this nb (PE slack under the exp stream)
                m2ps = psum.tile([128, 1024], f32, tag="big", name="m2ps",
                                 bufs=2)
                for pc in range(2):
                    for ci in range(NCC):
                        nc.tensor.matmul(
                            m2ps[:, pc * 512:(pc + 1) * 512],
                            m2r[ci][:, pc * 128:(pc + 1) * 128],
                            fmc[ci][nb][:],
                            start=(ci == 0), stop=(ci == NCC - 1))
                nc.vector.tensor_copy(s2f[0][:, ns], m2ps[:, 0:512])
                nc.vector.tensor_copy(s2f[1][:, ns], m2ps[:, 512:1024])

                at = sb.tile([65, 512], f32, tag="at", name="at", bufs=2)
                nc.vector.tensor_copy(at[:], pv[:])
                tbt = sb.tile([128, 256], bf16, tag="tb", name="tb", bufs=2)
                sa_norm[nb] = (at, tbt)
            normalize_rest(NB - 1, *sa_norm.pop(NB - 1), sc1, 0)

            # ---------------- MoCA: s2 = I@s2f + M2b@lat, 1-lag pipeline ----
            moca = {}
            for nb in range(NB + 1):
                if nb < NB:
                    ns = slice(nb * 512, (nb + 1) * 512)
                    s2 = psum.tile([128, 1024], f32, tag="big", name="s2",
                                   bufs=2)
                    for pc in range(2):
                        nc.tensor.matmul(
                            s2[:, pc * 512:(pc + 1) * 512],
                            id128r[:],
                            s2f[pc][:, ns],
                            start=True, stop=False)
                        nc.tensor.matmul(
                            s2[:, pc * 512:(pc + 1) * 512],
                            m2bb[:, pc * 128:(pc + 1) * 128],
                            latcat[0:64, ns],
                            start=False, stop=True)
                    p2t = sb.tile([128, 1024], bf16, tag="p2t", name="p2t",
                                  bufs=3)
                    nc.scalar.activation(p2t[:], s2[:], AFT.Exp)
                    moca[nb] = p2t
                if nb >= 1:
                    nbl = nb - 1
                    p2t = moca.pop(nbl)
                    pv2 = psum.tile([65, 512], f32, tag="pv", name="pv2",
                                    bufs=2, padded_shape=[128, 512])
                    for pc in range(2):
                        nc.tensor.matmul(pv2[:], p2w[:, pc * 65:(pc + 1) * 65],
                                         p2t[:, pc * 512:(pc + 1) * 512],
                                         start=(pc == 0), stop=(pc == 1))
                    at2 = sb.tile([65, 512], f32, tag="at", name="at2", bufs=2)
                    nc.vector.tensor_copy(at2[:], pv2[:])
                    tb2 = sb.tile([128, 256], bf16, tag="tb", name="tb2",
                                  bufs=2)
                    normalize_rest(nbl, at2, tb2, sc2, 64)

            # ---------------- tail: out = [wosa|womo]@[lat;lat2] + fm ----
            for nb in range(NB):
                ns = slice(nb * 512, (nb + 1) * 512)
                for g2 in range(2):
                    oc = psum.tile([128, 1024], f32, tag="big", name="oc",
                                   bufs=2)
                    ob = sb.tile([128, 1024], f32, tag="ob", name="ob", bufs=2)
                    for h in range(2):
                        cc = g2 * 2 + h
                        nc.tensor.matmul(oc[:, h * 512:(h + 1) * 512],
                                         wcatb[:, cc * 128:(cc + 1) * 128],
                                         latcat[:, ns],
                                         start=True, stop=True)
                        nc.vector.tensor_add(ob[:, h * 512:(h + 1) * 512],
                                             oc[:, h * 512:(h + 1) * 512],
                                             fmc[cc][nb][:])
                    ov = out_d[g2 * 256:(g2 + 1) * 256, ns].rearrange(
                        "(u p) c -> p u c", u=2)
                    ob_view = ob[:].rearrange("p (u c) -> p u c", u=2)
                    nc.sync.dma_start(ov, ob_view)

    nc.compile()
    return nc


def _get_runner(reps=1):
    """Build the Bass program once and return a cached jitted SPMD callable."""
    key = ("runner", reps)
    if key in _STATE:
        return _STATE[key]

    import jax
    import numpy as np
    from jax.experimental.shard_map import shard_map
    from jax.sharding import Mesh, PartitionSpec
    import concourse.mybir as mybir
    from concourse import bass2jax

    nc = _build_program(reps=reps)
    bass2jax.install_neuronx_cc_hook()

    partition_name = (nc.partition_id_tensor.name
                      if nc.partition_id_tensor else None)
    in_names, out_names, out_avals, zero_shapes = [], [], [], []
    for alloc in nc.m.functions[0].allocations:
        if not isinstance(alloc, mybir.MemoryLocationSet):
            continue
        name = alloc.memorylocations[0].name
        if alloc.kind == "ExternalInput":
            if name != partition_name:
                in_names.append(name)
        elif alloc.kind == "ExternalOutput":
            out_names.append(name)
            shape = tuple(alloc.tensor_shape)
            dtype = mybir.dt.np(alloc.dtype)
            out_avals.append(jax.core.ShapedArray(shape, dtype))
            zero_shapes.append((shape, dtype))
    n_params = len(in_names)
    all_in_names = list(in_names) + list(out_names)
    if partition_name is not None:
        all_in_names.append(partition_name)

    def _body(*args):
        operands = list(args)
        if partition_name is not None:
            operands.append(bass2jax.partition_id_tensor())
        outs = bass2jax._bass_exec_p.bind(
            *operands,
            out_avals=tuple(out_avals),
            in_names=tuple(all_in_names),
            out_names=tuple(out_names),
            lowering_input_output_aliases=(),
            sim_require_finite=True,
            sim_require_nnan=True,
            nc=nc,
        )
        return tuple(outs)

    devices = jax.devices()[:N_CORES]
    mesh = Mesh(np.asarray(devices), ("core",))
    n_outs = len(out_names)
    donate = tuple(range(n_params, n_params + n_outs))
    sharded = jax.jit(
        shard_map(_body, mesh=mesh,
                  in_specs=(PartitionSpec("core"),) * (n_params + n_outs),
                  out_specs=(PartitionSpec("core"),) * n_outs,
                  check_rep=False),
        donate_argnums=donate, keep_unused=True)

    runner = {
        "nc": nc, "sharded": sharded, "in_names": in_names,
        "out_names": out_names, "zero_shapes": zero_shapes,
        "n_params": n_params,
    }
    _STATE[key] = runner
    return runner


def _prep_in_maps(feature_map, concepts, w_theta, w_phi, w_g, w_o,
                  gamma_sa, gamma_moca):
    import ml_dtypes  # noqa: F401  (bf16 host casts if ever needed)

    feature_map = np.asarray(feature_map, dtype=np.float32)
    concepts = np.asarray(concepts, dtype=np.float32)
    w_theta = np.asarray(w_theta, dtype=np.float32)
    w_phi = np.asarray(w_phi, dtype=np.float32)
    w_g = np.asarray(w_g, dtype=np.float32)
    w_o = np.asarray(w_o, dtype=np.float32)
    gamma_sa = np.float32(gamma_sa)
    gamma_moca = np.float32(gamma_moca)

    gain = np.float32(1.0 / np.sqrt(C))
    gain_o = np.float32(1.0 / np.sqrt(L))

    wth_t = w_theta.T * gain                                        # [C, L]
    wph_t = w_phi.T * gain
    wthdup = np.ascontiguousarray(np.concatenate([wth_t, wth_t], axis=1))
    wphdup = np.ascontiguousarray(np.concatenate([wph_t, wph_t], axis=1))
    wg_t = np.ascontiguousarray(w_g.T * gain)                       # [C, L]
    m2 = concepts @ (w_theta * gain)            # [P, C]
    m2t = np.ascontiguousarray(m2.T)            # [C, P]
    m2b = (gamma_sa * gain_o) * (m2 @ w_o)      # [P, L]
    m2bt = np.ascontiguousarray(m2b.T)          # [L, P]
    wosa = w_o.T * (gain_o * gamma_sa)          # [L, C]
    womo = w_o.T * (gain_o * gamma_moca)        # [L, C]
    wcat = np.ascontiguousarray(np.concatenate([wosa, womo], axis=0))
    phiT = np.ascontiguousarray(concepts)                           # [P, L]
    fm_flat = feature_map.reshape(B, C, HW)

    in_maps = []
    for b in range(N_CORES):
        in_maps.append({
            "fm": np.ascontiguousarray(fm_flat[b]),
            "wthdup": wthdup, "wphdup": wphdup, "wg": wg_t,
            "m2t": m2t, "m2bt": m2bt, "wcat": wcat, "phiT": phiT,
        })
    return in_maps


def _run(in_maps):
    r = _get_runner()
    n_params = r["n_params"]
    concat_in = [
        np.concatenate([np.asarray(in_maps[c][name])
                        for c in range(N_CORES)], axis=0)
        for name in r["in_names"]
    ]
    concat_zeros = [np.zeros((N_CORES * s[0], *s[1:]), d)
                    for (s, d) in r["zero_shapes"]]
    out_arrs = r["sharded"](*concat_in, *concat_zeros)
    per_core = []
    for c in range(N_CORES):
        per_core.append({
            name: np.asarray(out_arrs[i]).reshape(
                N_CORES, *r["zero_shapes"][i][0])[c]
            for i, name in enumerate(r["out_names"])
        })
    return per_core


def kernel(feature_map, concepts, w_theta, w_phi, w_g, w_o,
           gamma_sa, gamma_moca):
    in_maps = _prep_in_maps(feature_map, concepts, w_theta, w_phi, w_g, w_o,
                            gamma_sa, gamma_moca)
    per_core = _run(in_maps)
    out = np.stack([per_core[b]["out"].reshape(C, H, W)
                    for b in range(B)], axis=0)
    return out.astype(np.float32)


# revision 18
# speedup vs baseline: 32.9820x; 1.9406x over previous
"""Trainium2 Bass kernel for nn_MoCA (self-attention + momentum concept attention).

Sharding: pure data parallel — batch dim (B=8) sharded 1 batch per NeuronCore,
weights/concept pool replicated. No collectives.

v2 — restructured around the ACT-engine exp roofline (~147 us of exp per core):
  * fm loaded as 32 per-(ci,nb) chunk tiles; convs pipeline behind the DMAs
    (per-chunk DVE f32->f32r rounding copies, off the ACT engine).
  * SA phase: ACT runs ONLY exps; all PSUM drains (th/ph/g/at/s2f) are DVE.
    ST/PV software pipeline (LAG=2) with quadrant-packed K=64 score matmuls.
  * s2_fm = M2 @ fm (the enc-conv/concept matmul folded with the fm part of
    sa_out) is computed inside the SA window using PE slack, into s2f SBUF.
  * sa_out is never materialized: s2 = I@s2f + (gamma_sa*M2@w_o)@lat, and the
    final output is one merged conv out = [wosa|womo] @ [lat;lat2] + fm.
  * normalize path: PV emits [attnT|denom]; PE-transpose back to [n, l],
    multiply by reciprocal denominator, one 3D-AP DMA to a DRAM scratch
    [HW, L] and re-read as [L, HW] (the torch .view memory reinterpret).
  * PSUM: tags big(2 banks x3) / pv(1 x1) / tt(1 x1) = 8 banks.
"""
import sys

if '/opt/trn_rl_repo' not in sys.path:
    sys.path.insert(0, '/opt/trn_rl_repo')

import numpy as np

C, L, H, W, P = 512, 64, 64, 64, 256
HW = H * W
B = 8
N_CORES = 8

_STATE: dict = {}


def _build_program(reps=1, num_devices=N_CORES):
    import concourse.bass as bass
    import concourse.bacc as bacc
    import concourse.mybir as mybir
    from concourse import tile
    from concourse.masks import make_identity

    dt = mybir.dt
    AFT = mybir.ActivationFunctionType
    f32, f32r, bf16 = dt.float32, dt.float32r, dt.bfloat16

    nc = bacc.Bacc("TRN2", target_bir_lowering=False, debug=False,
                   enable_asserts=False, num_devices=num_devices)

    fm_d = nc.dram_tensor("fm", [C, HW], f32, kind="ExternalInput").ap()
    wthdup_d = nc.dram_tensor("wthdup", [C, 128], f32, kind="ExternalInput").ap()
    wphdup_d = nc.dram_tensor("wphdup", [C, 128], f32, kind="ExternalInput").ap()
    wg_d = nc.dram_tensor("wg", [C, L], f32, kind="ExternalInput").ap()
    m2t_d = nc.dram_tensor("m2t", [C, P], f32, kind="ExternalInput").ap()
    m2bt_d = nc.dram_tensor("m2bt", [L, P], f32, kind="ExternalInput").ap()
    wcat_d = nc.dram_tensor("wcat", [128, C], f32, kind="ExternalInput").ap()
    phiT_d = nc.dram_tensor("phiT", [P, L], f32, kind="ExternalInput").ap()
    out_d = nc.dram_tensor("out", [C, HW], f32, kind="ExternalOutput").ap()

    NB = HW // 512          # 8 n-blocks of 512
    NM = HW // 128          # 32 m-chunks of 128
    NCC = C // 128          # 4 channel chunks
    NPAIR = NM // 2         # 16 pair-steps per n-block
    LAG = 2

    with tile.TileContext(nc) as tc:
      for _rep in range(reps):
        with tc.tile_pool(name="sb", bufs=1) as sb, \
             tc.tile_pool(name="dram", bufs=1, space="DRAM") as dp, \
             tc.tile_pool(name="ps", bufs=1, space="PSUM") as psum:

            sc1 = dp.tile([HW, L], bf16, tag="sc1", name="sc1")
            sc2 = dp.tile([HW, L], bf16, tag="sc2", name="sc2")

            # ---------------- persistent tiles ----------------
            fmc = [[sb.tile([128, 512], f32r, tag=f"fmc{ci}_{nb}",
                            name=f"fmc{ci}_{nb}")
                    for nb in range(NB)] for ci in range(NCC)]
            th = sb.tile([128, HW], f32r, tag="th", name="th")
            ph = sb.tile([128, HW], f32r, tag="ph", name="ph")
            g_sb = sb.tile([L, HW], bf16, tag="g_sb", name="g_sb")
            gto = sb.tile([128, NM * 65], bf16, tag="gto", name="gto")
            s2f = [sb.tile([128, HW], f32r, tag=f"s2f{i}", name=f"s2f{i}")
                   for i in range(2)]
            latcat = sb.tile([128, HW], bf16, tag="latcat", name="latcat")

            wthr = [sb.tile([128, 128], f32r, tag=f"wthr{i}", name=f"wthr{i}")
                    for i in range(NCC)]
            wphr = [sb.tile([128, 128], f32r, tag=f"wphr{i}", name=f"wphr{i}")
                    for i in range(NCC)]
            wgr = [sb.tile([128, L], f32r, tag=f"wgr{i}", name=f"wgr{i}")
                   for i in range(NCC)]
            m2r = [sb.tile([128, P], f32r, tag=f"m2r{i}", name=f"m2r{i}")
                   for i in range(NCC)]
            m2bb = sb.tile([L, P], bf16, tag="m2bb", name="m2bb")
            wcatb = sb.tile([128, C], bf16, tag="wcatb", name="wcatb")
            p2w = sb.tile([128, 2 * 65], bf16, tag="p2w", name="p2w")
            id64 = sb.tile([64, 64], bf16, tag="id64", name="id64")
            id65 = sb.tile([65, 65], f32, tag="id65", name="id65")
            id128 = sb.tile([128, 128], f32, tag="id128", name="id128")
            id128r = sb.tile([128, 128], f32r, tag="id128r", name="id128r")

            make_identity(nc, id64[:])
            make_identity(nc, id65[:])
            make_identity(nc, id128[:])
            nc.vector.tensor_copy(id128r[:], id128[:])
            nc.vector.memset(p2w[:], 1.0)
            nc.vector.memset(gto[:], 1.0)

            # ---------------- weight loads (small, first) ----------------
            for ci in range(NCC):
                tw = sb.tile([128, 128], f32, tag="tw", name="tw", bufs=2)
                nc.sync.dma_start(tw[:], wthdup_d[ci * 128:(ci + 1) * 128, :])
                nc.vector.tensor_copy(wthr[ci][:], tw[:])
                tw = sb.tile([128, 128], f32, tag="tw", name="tw", bufs=2)
                nc.sync.dma_start(tw[:], wphdup_d[ci * 128:(ci + 1) * 128, :])
                nc.vector.tensor_copy(wphr[ci][:], tw[:])
                tw = sb.tile([128, L], f32, tag="twg", name="twg", bufs=2)
                nc.sync.dma_start(tw[:], wg_d[ci * 128:(ci + 1) * 128, :])
                nc.vector.tensor_copy(wgr[ci][:], tw[:])
                tw = sb.tile([128, P], f32, tag="twm", name="twm", bufs=2)
                nc.sync.dma_start(tw[:], m2t_d[ci * 128:(ci + 1) * 128, :])
                nc.vector.tensor_copy(m2r[ci][:], tw[:])
            twc = sb.tile([128, C], f32, tag="twc", name="twc")
            nc.sync.dma_start(twc[:], wcat_d[:])
            nc.vector.tensor_copy(wcatb[:], twc[:])
            tmb = sb.tile([L, P], f32, tag="tmb", name="tmb")
            nc.sync.dma_start(tmb[:], m2bt_d[:])
            nc.vector.tensor_copy(m2bb[:], tmb[:])
            for pc in range(2):
                tph = sb.tile([128, L], f32, tag="tph", name="tph", bufs=2)
                nc.sync.dma_start(tph[:], phiT_d[pc * 128:(pc + 1) * 128, :])
                nc.vector.tensor_copy(p2w[:, pc * 65:pc * 65 + 64], tph[:])

            # ---------------- fm chunk loads (nb-major) + f32r rounding ----
            for nb in range(NB):
                ns = slice(nb * 512, (nb + 1) * 512)
                for ci in range(NCC):
                    tf = sb.tile([128, 512], f32, tag="tf", name="tf", bufs=2)
                    nc.sync.dma_start(tf[:], fm_d[ci * 128:(ci + 1) * 128, ns])
                    nc.vector.tensor_copy(fmc[ci][nb][:], tf[:])

            # ------- head: th/ph/g convs + M2@fm (s2f) + 1-lagged gT -------
            def gtrans(nbl, k):
                mc = nbl * 4 + k
                tp = psum.tile([128, 64], bf16, tag="tt", name="gtp",
                               bufs=1, padded_shape=[128, 1024])
                nc.tensor.transpose(tp[:], g_sb[:, mc * 128:(mc + 1) * 128],
                                    id64[:])
                nc.vector.tensor_copy(gto[:, mc * 65:mc * 65 + 64], tp[:])

            for nb in range(NB):
                ns = slice(nb * 512, (nb + 1) * 512)
                thph = psum.tile([128, 1024], f32, tag="big", name="thph",
                                 bufs=3)
                for ci in range(NCC):
                    nc.tensor.matmul(thph[:, 0:512],
                                     wthr[ci][:],
                                     fmc[ci][nb][:],
                                     start=(ci == 0), stop=(ci == NCC - 1))
                if nb >= 1:
                    gtrans(nb - 1, 0)
                    gtrans(nb - 1, 1)
                for ci in range(NCC):
                    nc.tensor.matmul(thph[:, 512:1024],
                                     wphr[ci][:],
                                     fmc[ci][nb][:],
                                     start=(ci == 0), stop=(ci == NCC - 1))
                nc.vector.tensor_copy(th[:, ns], thph[:, 0:512])
                nc.vector.tensor_copy(ph[:, ns], thph[:, 512:1024])

                psg = psum.tile([L, 512], f32, tag="big", name="g_ps", bufs=3,
                                padded_shape=[128, 1024])
                for ci in range(NCC):
                    nc.tensor.matmul(psg[:], wgr[ci][:],
                                     fmc[ci][nb][:],
                                     start=(ci == 0), stop=(ci == NCC - 1))
                if nb >= 1:
                    gtrans(nb - 1, 2)
                    gtrans(nb - 1, 3)
                nc.vector.tensor_copy(g_sb[:, ns], psg[:])

                m2ps = psum.tile([128, 1024], f32, tag="big", name="m2ps",
                                 bufs=3)
                for pc in range(2):
                    for ci in range(NCC):
                        nc.tensor.matmul(
                            m2ps[:, pc * 512:(pc + 1) * 512],
                            m2r[ci][:, pc * 128:(pc + 1) * 128],
                            fmc[ci][nb][:],
                            start=(ci == 0), stop=(ci == NCC - 1))
                nc.vector.tensor_copy(s2f[0][:, ns], m2ps[:, 0:512])
                nc.vector.tensor_copy(s2f[1][:, ns], m2ps[:, 512:1024])
            for k in range(4):
                gtrans(NB - 1, k)

            # ------------ SA with 1-nb-lagged, spread-out normalize ------------
            def norm_step(nbl, at, tbt, k):
                """One attnT block: PE-transpose, reciprocal-denom scale."""
                tp = psum.tile([128, 65], f32, tag="tt", name="tt",
                               bufs=1, padded_shape=[128, 512])
                nc.tensor.transpose(tp[:], at[:, k * 128:(k + 1) * 128],
                                    id65[:])
                rc = sb.tile([128, 1], f32, tag="rc", name="rc", bufs=2)
                nc.vector.reciprocal(rc[:], tp[:, 64:65])
                nc.vector.tensor_scalar_mul(tbt[:, k * 64:(k + 1) * 64],
                                            tp[:, 0:64], rc[:])

            def norm_dma(nbl, tbt, scr, dst_row0):
                """1 DMA to DRAM scratch, 1 DMA back as [L, HW] rows."""
                scr_view = scr[nbl * 512:(nbl + 1) * 512, :].rearrange(
                    "(k p) c -> p k c", k=4)
                tbt_view = tbt[:].rearrange("p (k c) -> p k c", k=4)
                nc.sync.dma_start(scr_view, tbt_view)
                lat_view = scr[:].rearrange("(a b) c -> a (b c)", a=L)
                nc.sync.dma_start(latcat[dst_row0 + nbl * 8:
                                         dst_row0 + (nbl + 1) * 8, :],
                                  lat_view[nbl * 8:(nbl + 1) * 8, :])

            sa_norm = {}
            for nb in range(NB):
                ns = slice(nb * 512, (nb + 1) * 512)
                pv = psum.tile([65, 512], f32, tag="pv", name="pv", bufs=1,
                               padded_shape=[128, 512])
                pts = {}
                for j in range(NPAIR + LAG):
                    if j < NPAIR:
                        st = psum.tile([128, 1024], f32, tag="big", name="st",
                                       bufs=3)
                        for h in range(2):
                            mc = 2 * j + h
                            hp = slice(64 * h, 64 * h + 64)
                            nc.tensor.matmul(
                                st[:, h * 512:(h + 1) * 512],
                                ph[hp, mc * 128:(mc + 1) * 128],
                                th[hp, ns],
                                start=True, stop=True,
                                tile_position=(64 * h, 0))
                        ptt = sb.tile([128, 1024], bf16, tag="pt", name="pt",
                                      bufs=LAG + 2)
                        nc.scalar.activation(ptt[:], st[:], AFT.Exp)
                        pts[j] = ptt
                    if nb >= 1 and 1 <= j <= 4:
                        norm_step(nb - 1, *sa_norm[nb - 1], j - 1)
                    if j == 5 and nb >= 1:
                        norm_dma(nb - 1, sa_norm.pop(nb - 1)[1], sc1, 0)
                    if j >= LAG:
                        jj = j - LAG
                        ptt = pts.pop(jj)
                        for h in range(2):
                            mc = 2 * jj + h
                            nc.tensor.matmul(
                                pv[:], gto[:, mc * 65:(mc + 1) * 65],
                                ptt[:, h * 512:(h + 1) * 512],
                                start=(mc == 0), stop=(mc == NM - 1))
                at = sb.tile([65, 512], f32, tag="at", name="at", bufs=2)
                nc.vector.tensor_copy(at[:], pv[:])
                tbt = sb.tile([128, 256], bf16, tag="tb", name="tb", bufs=2)
                sa_norm[nb] = (at, tbt)
            for k in range(4):
                norm_step(NB - 1, *sa_norm[NB - 1], k)
            norm_dma(NB - 1, sa_norm.pop(NB - 1)[1], sc1, 0)

            # ------- MoCA: s2 = I@s2f + M2b@lat, 1-lag + spread normalize ----
            moca = {}
            mo_norm = {}
            for nb in range(NB + 2):
                if nb < NB:
                    ns = slice(nb * 512, (nb + 1) * 512)
                    s2 = psum.tile([128, 1024], f32, tag="big", name="s2",
                                   bufs=3)
                    for pc in range(2):
                        nc.tensor.matmul(
                            s2[:, pc * 512:(pc + 1) * 512],
                            id128r[:],
                            s2f[pc][:, ns],
                            start=True, stop=False)
                        nc.tensor.matmul(
                            s2[:, pc * 512:(pc + 1) * 512],
                            m2bb[:, pc * 128:(pc + 1) * 128],
                            latcat[0:64, ns],
                            start=False, stop=True)
                    p2t = sb.tile([128, 1024], bf16, tag="p2t", name="p2t",
                                  bufs=3)
                    nc.scalar.activation(p2t[:], s2[:], AFT.Exp)
                    moca[nb] = p2t
                if nb >= 2:
                    nbll = nb - 2
                    at2, tb2 = mo_norm.pop(nbll)
                    for k in range(4):
                        norm_step(nbll, at2, tb2, k)
                    norm_dma(nbll, tb2, sc2, 64)
                if 1 <= nb <= NB:
                    nbl = nb - 1
                    p2t = moca.pop(nbl)
                    pv2 = psum.tile([65, 512], f32, tag="pv", name="pv2",
                                    bufs=1, padded_shape=[128, 512])
                    for pc in range(2):
                        nc.tensor.matmul(pv2[:], p2w[:, pc * 65:(pc + 1) * 65],
                                         p2t[:, pc * 512:(pc + 1) * 512],
                                         start=(pc == 0), stop=(pc == 1))
                    at2 = sb.tile([65, 512], f32, tag="at", name="at2", bufs=2)
                    nc.vector.tensor_copy(at2[:], pv2[:])
                    tb2 = sb.tile([128, 256], bf16, tag="tb", name="tb2",
                                  bufs=2)
                    mo_norm[nbl] = (at2, tb2)

            # ---------------- tail: out = [wosa|womo]@[lat;lat2] + fm ----
            for nb in range(NB):
                ns = slice(nb * 512, (nb + 1) * 512)
                for g2 in range(2):
                    oc = psum.tile([128, 1024], f32, tag="big", name="oc",
                                   bufs=3)
                    ob = sb.tile([128, 1024], f32, tag="ob", name="ob", bufs=2)
                    for h in range(2):
                        cc = g2 * 2 + h
                        nc.tensor.matmul(oc[:, h * 512:(h + 1) * 512],
                                         wcatb[:, cc * 128:(cc + 1) * 128],
                                         latcat[:, ns],
                                         start=True, stop=True)
                        nc.vector.tensor_add(ob[:, h * 512:(h + 1) * 512],
                                             oc[:, h * 512:(h + 1) * 512],
                                             fmc[cc][nb][:])
                    ov = out_d[g2 * 256:(g2 + 1) * 256, ns].rearrange(
                        "(u p) c -> p u c", u=2)
                    ob_view = ob[:].rearrange("p (u c) -> p u c", u=2)
                    nc.sync.dma_start(ov, ob_view)

    nc.compile()
    return nc


def _get_runner(reps=1):
    """Build the Bass program once and return a cached jitted SPMD callable."""
    key = ("runner", reps)
    if key in _STATE:
        return _STATE[key]

    import jax
    import numpy as np
    from jax.experimental.shard_map import shard_map
    from jax.sharding import Mesh, PartitionSpec
    import concourse.mybir as mybir
    from concourse import bass2jax

    nc = _build_program(reps=reps)
    bass2jax.install_neuronx_cc_hook()

    partition_name = (nc.partition_id_tensor.name
                      if nc.partition_id_tensor else None)
    in_names, out_names, out_avals, zero_shapes = [], [], [], []
    for alloc in nc.m.functions[0].allocations:
        if not isinstance(alloc, mybir.MemoryLocationSet):
            continue
        name = alloc.memorylocations[0].name
        if alloc.kind == "ExternalInput":
            if name != partition_name:
                in_names.append(name)
        elif alloc.kind == "ExternalOutput":
            out_names.append(name)
            shape = tuple(alloc.tensor_shape)
            dtype = mybir.dt.np(alloc.dtype)
            out_avals.append(jax.core.ShapedArray(shape, dtype))
            zero_shapes.append((shape, dtype))
    n_params = len(in_names)
    all_in_names = list(in_names) + list(out_names)
    if partition_name is not None:
        all_in_names.append(partition_name)

    def _body(*args):
        operands = list(args)
        if partition_name is not None:
            operands.append(bass2jax.partition_id_tensor())
        outs = bass2jax._bass_exec_p.bind(
            *operands,
            out_avals=tuple(out_avals),
            in_names=tuple(all_in_names),
            out_names=tuple(out_names),
            lowering_input_output_aliases=(),
            sim_require_finite=True,
            sim_require_nnan=True,
            nc=nc,
        )
        return tuple(outs)

    devices = jax.devices()[:N_CORES]
    mesh = Mesh(np.asarray(devices), ("core",))
    n_outs = len(out_names)
    donate = tuple(range(n_params, n_params + n_outs))
    sharded = jax.jit(
        shard_map(_body, mesh=mesh,
                  in_specs=(PartitionSpec("core"),) * (n_params + n_outs),
                  out_specs=(PartitionSpec("core"),) * n_outs,
                  check_rep=False),
        donate_argnums=donate, keep_unused=True)

    runner = {
        "nc": nc, "sharded": sharded, "in_names": in_names,
        "out_names": out_names, "zero_shapes": zero_shapes,
        "n_params": n_params,
    }
    _STATE[key] = runner
    return runner


def _prep_in_maps(feature_map, concepts, w_theta, w_phi, w_g, w_o,
                  gamma_sa, gamma_moca):
    import ml_dtypes  # noqa: F401  (bf16 host casts if ever needed)

    feature_map = np.asarray(feature_map, dtype=np.float32)
    concepts = np.asarray(concepts, dtype=np.float32)
    w_theta = np.asarray(w_theta, dtype=np.float32)
    w_phi = np.asarray(w_phi, dtype=np.float32)
    w_g = np.asarray(w_g, dtype=np.float32)
    w_o = np.asarray(w_o, dtype=np.float32)
    gamma_sa = np.float32(gamma_sa)
    gamma_moca = np.float32(gamma_moca)

    gain = np.float32(1.0 / np.sqrt(C))
    gain_o = np.float32(1.0 / np.sqrt(L))

    wth_t = w_theta.T * gain                                        # [C, L]
    wph_t = w_phi.T * gain
    wthdup = np.ascontiguousarray(np.concatenate([wth_t, wth_t], axis=1))
    wphdup = np.ascontiguousarray(np.concatenate([wph_t, wph_t], axis=1))
    wg_t = np.ascontiguousarray(w_g.T * gain)                       # [C, L]
    m2 = concepts @ (w_theta * gain)            # [P, C]
    m2t = np.ascontiguousarray(m2.T)            # [C, P]
    m2b = (gamma_sa * gain_o) * (m2 @ w_o)      # [P, L]
    m2bt = np.ascontiguousarray(m2b.T)          # [L, P]
    wosa = w_o.T * (gain_o * gamma_sa)          # [L, C]
    womo = w_o.T * (gain_o * gamma_moca)        # [L, C]
    wcat = np.ascontiguousarray(np.concatenate([wosa, womo], axis=0))
    phiT = np.ascontiguousarray(concepts)                           # [P, L]
    fm_flat = feature_map.reshape(B, C, HW)

    in_maps = []
    for b in range(N_CORES):
        in_maps.append({
            "fm": np.ascontiguousarray(fm_flat[b]),
            "wthdup": wthdup, "wphdup": wphdup, "wg": wg_t,
            "m2t": m2t, "m2bt": m2bt, "wcat": wcat, "phiT": phiT,
        })
    return in_maps


def _run(in_maps):
    r = _get_runner()
    n_params = r["n_params"]
    concat_in = [
        np.concatenate([np.asarray(in_maps[c][name])
                        for c in range(N_CORES)], axis=0)
        for name in r["in_names"]
    ]
    concat_zeros = [np.zeros((N_CORES * s[0], *s[1:]), d)
                    for (s, d) in r["zero_shapes"]]
    out_arrs = r["sharded"](*concat_in, *concat_zeros)
    per_core = []
    for c in range(N_CORES):
        per_core.append({
            name: np.asarray(out_arrs[i]).reshape(
                N_CORES, *r["zero_shapes"][i][0])[c]
            for i, name in enumerate(r["out_names"])
        })
    return per_core


def kernel(feature_map, concepts, w_theta, w_phi, w_g, w_o,
           gamma_sa, gamma_moca):
    in_maps = _prep_in_maps(feature_map, concepts, w_theta, w_phi, w_g, w_o,
                            gamma_sa, gamma_moca)
    per_core = _run(in_maps)
    out = np.stack([per_core[b]["out"].reshape(C, H, W)
                    for b in range(B)], axis=0)
    return out.astype(np.float32)


# revision 19
# speedup vs baseline: 38.0209x; 1.1528x over previous
"""Trainium2 Bass kernel for nn_MoCA (self-attention + momentum concept attention).

Sharding: pure data parallel — batch dim (B=8) sharded 1 batch per NeuronCore,
weights/concept pool replicated. No collectives.

v4 — restructured around the ACT-engine exp roofline (~147 us of exp per core):
  * whole conv/score path in bf16 (convs, th/ph, s2f, weights); fm also kept
    in f32 chunk tiles for the exact final residual add.
  * SA phase: ACT runs ONLY exps; all PSUM drains (th/ph/g/at/s2f) are DVE.
    ST/PV software pipeline (LAG=2) with quadrant-packed K=64 score matmuls.
  * s2_fm = M2 @ fm (the enc-conv/concept matmul folded with the fm part of
    sa_out) is computed inside the SA window using PE slack, into s2f SBUF.
  * sa_out is never materialized: s2 = I@s2f + (gamma_sa*M2@w_o)@lat, and the
    final output is one merged conv out = [wosa|womo] @ [lat;lat2] + fm.
  * normalize path: PV emits [attnT|denom]; PE-transpose back to [n, l],
    multiply by reciprocal denominator, one 3D-AP DMA to a DRAM scratch
    [HW, L] and re-read as [L, HW] (the torch .view memory reinterpret).
  * PSUM: tags big(2 banks x3) / pv(1 x1) / tt(1 x1) = 8 banks;\n    trailing normalize chains alternate tt/pv to pipeline.
"""
import sys

if '/opt/trn_rl_repo' not in sys.path:
    sys.path.insert(0, '/opt/trn_rl_repo')

import numpy as np

C, L, H, W, P = 512, 64, 64, 64, 256
HW = H * W
B = 8
N_CORES = 8

_STATE: dict = {}


def _build_program(reps=1, num_devices=N_CORES):
    import concourse.bass as bass
    import concourse.bacc as bacc
    import concourse.mybir as mybir
    from concourse import tile
    from concourse.masks import make_identity

    dt = mybir.dt
    AFT = mybir.ActivationFunctionType
    f32, bf16 = dt.float32, dt.bfloat16

    nc = bacc.Bacc("TRN2", target_bir_lowering=False, debug=False,
                   enable_asserts=False, num_devices=num_devices)

    fm_d = nc.dram_tensor("fm", [C, HW], f32, kind="ExternalInput").ap()
    wthdup_d = nc.dram_tensor("wthdup", [C, 128], f32, kind="ExternalInput").ap()
    wphdup_d = nc.dram_tensor("wphdup", [C, 128], f32, kind="ExternalInput").ap()
    wg_d = nc.dram_tensor("wg", [C, L], f32, kind="ExternalInput").ap()
    m2t_d = nc.dram_tensor("m2t", [C, P], f32, kind="ExternalInput").ap()
    m2bt_d = nc.dram_tensor("m2bt", [L, P], f32, kind="ExternalInput").ap()
    wcat_d = nc.dram_tensor("wcat", [128, C], f32, kind="ExternalInput").ap()
    phiT_d = nc.dram_tensor("phiT", [P, L], f32, kind="ExternalInput").ap()
    out_d = nc.dram_tensor("out", [C, HW], f32, kind="ExternalOutput").ap()

    NB = HW // 512          # 8 n-blocks of 512
    NM = HW // 128          # 32 m-chunks of 128
    NCC = C // 128          # 4 channel chunks
    NPAIR = NM // 2         # 16 pair-steps per n-block
    LAG = 2

    with tile.TileContext(nc) as tc:
      for _rep in range(reps):
        with tc.tile_pool(name="sb", bufs=1) as sb, \
             tc.tile_pool(name="dram", bufs=1, space="DRAM") as dp, \
             tc.tile_pool(name="ps", bufs=1, space="PSUM") as psum:

            sc1 = dp.tile([HW, L], bf16, tag="sc1", name="sc1")
            sc2 = dp.tile([HW, L], bf16, tag="sc2", name="sc2")

            # ---------------- persistent tiles ----------------
            # fm kept twice: f32 chunks for the final residual add, bf16
            # chunks for all matmuls.
            fmf = [[sb.tile([128, 512], f32, tag=f"fmf{ci}_{nb}",
                            name=f"fmf{ci}_{nb}")
                    for nb in range(NB)] for ci in range(NCC)]
            fmc = [[sb.tile([128, 512], bf16, tag=f"fmc{ci}_{nb}",
                            name=f"fmc{ci}_{nb}")
                    for nb in range(NB)] for ci in range(NCC)]
            th = sb.tile([128, HW], bf16, tag="th", name="th")
            ph = sb.tile([128, HW], bf16, tag="ph", name="ph")
            g_sb = sb.tile([L, HW], bf16, tag="g_sb", name="g_sb")
            gto = sb.tile([128, NM * 65], bf16, tag="gto", name="gto")
            s2f = [sb.tile([128, HW], bf16, tag=f"s2f{i}", name=f"s2f{i}")
                   for i in range(2)]
            latcat = sb.tile([128, HW], bf16, tag="latcat", name="latcat")

            wthr = [sb.tile([128, 128], bf16, tag=f"wthr{i}", name=f"wthr{i}")
                    for i in range(NCC)]
            wphr = [sb.tile([128, 128], bf16, tag=f"wphr{i}", name=f"wphr{i}")
                    for i in range(NCC)]
            wgr = [sb.tile([128, L], bf16, tag=f"wgr{i}", name=f"wgr{i}")
                   for i in range(NCC)]
            m2r = [sb.tile([128, P], bf16, tag=f"m2r{i}", name=f"m2r{i}")
                   for i in range(NCC)]
            m2bb = sb.tile([L, P], bf16, tag="m2bb", name="m2bb")
            wcatb = sb.tile([128, C], bf16, tag="wcatb", name="wcatb")
            p2w = sb.tile([128, 2 * 65], bf16, tag="p2w", name="p2w")
            id64 = sb.tile([64, 64], bf16, tag="id64", name="id64")
            id65 = sb.tile([65, 65], f32, tag="id65", name="id65")
            id128b = sb.tile([128, 128], bf16, tag="id128b", name="id128b")

            make_identity(nc, id64[:])
            make_identity(nc, id65[:])
            make_identity(nc, id128b[:])
            nc.vector.memset(p2w[:], 1.0)
            nc.vector.memset(gto[:], 1.0)

            # ---------------- weight loads (small, first) ----------------
            for ci in range(NCC):
                tw = sb.tile([128, 128], f32, tag="tw", name="tw", bufs=2)
                nc.sync.dma_start(tw[:], wthdup_d[ci * 128:(ci + 1) * 128, :])
                nc.vector.tensor_copy(wthr[ci][:], tw[:])
                tw = sb.tile([128, 128], f32, tag="tw", name="tw", bufs=2)
                nc.sync.dma_start(tw[:], wphdup_d[ci * 128:(ci + 1) * 128, :])
                nc.vector.tensor_copy(wphr[ci][:], tw[:])
                tw = sb.tile([128, L], f32, tag="twg", name="twg", bufs=2)
                nc.sync.dma_start(tw[:], wg_d[ci * 128:(ci + 1) * 128, :])
                nc.vector.tensor_copy(wgr[ci][:], tw[:])
                tw = sb.tile([128, P], f32, tag="twm", name="twm", bufs=2)
                nc.sync.dma_start(tw[:], m2t_d[ci * 128:(ci + 1) * 128, :])
                nc.vector.tensor_copy(m2r[ci][:], tw[:])
            twc = sb.tile([128, C], f32, tag="twc", name="twc")
            nc.sync.dma_start(twc[:], wcat_d[:])
            nc.vector.tensor_copy(wcatb[:], twc[:])
            tmb = sb.tile([L, P], f32, tag="tmb", name="tmb")
            nc.sync.dma_start(tmb[:], m2bt_d[:])
            nc.vector.tensor_copy(m2bb[:], tmb[:])
            for pc in range(2):
                tph = sb.tile([128, L], f32, tag="tph", name="tph", bufs=2)
                nc.sync.dma_start(tph[:], phiT_d[pc * 128:(pc + 1) * 128, :])
                nc.vector.tensor_copy(p2w[:, pc * 65:pc * 65 + 64], tph[:])

            # -------- fm chunk loads (nb-major) + bf16 shadow copies --------
            for nb in range(NB):
                ns = slice(nb * 512, (nb + 1) * 512)
                for ci in range(NCC):
                    nc.sync.dma_start(fmf[ci][nb][:],
                                      fm_d[ci * 128:(ci + 1) * 128, ns])
                    nc.vector.tensor_copy(fmc[ci][nb][:], fmf[ci][nb][:])

            # ------- head: th/ph/g convs + M2@fm (s2f) + 1-lagged gT -------
            def gtrans(nbl, k):
                mc = nbl * 4 + k
                tp = psum.tile([128, 64], bf16, tag="tt", name="gtp",
                               bufs=1, padded_shape=[128, 1024])
                nc.tensor.transpose(tp[:], g_sb[:, mc * 128:(mc + 1) * 128],
                                    id64[:])
                nc.vector.tensor_copy(gto[:, mc * 65:mc * 65 + 64], tp[:])

            for nb in range(NB):
                ns = slice(nb * 512, (nb + 1) * 512)
                thph = psum.tile([128, 1024], f32, tag="big", name="thph",
                                 bufs=3)
                for ci in range(NCC):
                    nc.tensor.matmul(thph[:, 0:512],
                                     wthr[ci][:],
                                     fmc[ci][nb][:],
                                     start=(ci == 0), stop=(ci == NCC - 1))
                if nb >= 1:
                    gtrans(nb - 1, 0)
                    gtrans(nb - 1, 1)
                for ci in range(NCC):
                    nc.tensor.matmul(thph[:, 512:1024],
                                     wphr[ci][:],
                                     fmc[ci][nb][:],
                                     start=(ci == 0), stop=(ci == NCC - 1))
                nc.vector.tensor_copy(th[:, ns], thph[:, 0:512])
                nc.vector.tensor_copy(ph[:, ns], thph[:, 512:1024])

                psg = psum.tile([L, 512], f32, tag="big", name="g_ps", bufs=3,
                                padded_shape=[128, 1024])
                for ci in range(NCC):
                    nc.tensor.matmul(psg[:], wgr[ci][:],
                                     fmc[ci][nb][:],
                                     start=(ci == 0), stop=(ci == NCC - 1))
                if nb >= 1:
                    gtrans(nb - 1, 2)
                    gtrans(nb - 1, 3)
                nc.vector.tensor_copy(g_sb[:, ns], psg[:])

                m2ps = psum.tile([128, 1024], f32, tag="big", name="m2ps",
                                 bufs=3)
                for pc in range(2):
                    for ci in range(NCC):
                        nc.tensor.matmul(
                            m2ps[:, pc * 512:(pc + 1) * 512],
                            m2r[ci][:, pc * 128:(pc + 1) * 128],
                            fmc[ci][nb][:],
                            start=(ci == 0), stop=(ci == NCC - 1))
                nc.vector.tensor_copy(s2f[0][:, ns], m2ps[:, 0:512])
                nc.vector.tensor_copy(s2f[1][:, ns], m2ps[:, 512:1024])
            for k in range(4):
                gtrans(NB - 1, k)

            # ------------ SA with 1-nb-lagged, spread-out normalize ------------
            def norm_step(nbl, at, tbt, k, tag="tt"):
                """One attnT block: PE-transpose, reciprocal-denom scale."""
                tp = psum.tile([128, 65], f32, tag=tag, name="tt",
                               bufs=1, padded_shape=[128, 512])
                nc.tensor.transpose(tp[:], at[:, k * 128:(k + 1) * 128],
                                    id65[:])
                rc = sb.tile([128, 1], f32, tag="rc", name="rc", bufs=2)
                nc.vector.reciprocal(rc[:], tp[:, 64:65])
                nc.vector.tensor_scalar_mul(tbt[:, k * 64:(k + 1) * 64],
                                            tp[:, 0:64], rc[:])

            def norm_dma(nbl, tbt, scr, dst_row0):
                """1 DMA to DRAM scratch, 1 DMA back as [L, HW] rows."""
                scr_view = scr[nbl * 512:(nbl + 1) * 512, :].rearrange(
                    "(k p) c -> p k c", k=4)
                tbt_view = tbt[:].rearrange("p (k c) -> p k c", k=4)
                nc.sync.dma_start(scr_view, tbt_view)
                lat_view = scr[:].rearrange("(a b) c -> a (b c)", a=L)
                nc.sync.dma_start(latcat[dst_row0 + nbl * 8:
                                         dst_row0 + (nbl + 1) * 8, :],
                                  lat_view[nbl * 8:(nbl + 1) * 8, :])

            sa_norm = {}
            for nb in range(NB):
                ns = slice(nb * 512, (nb + 1) * 512)
                pv = psum.tile([65, 512], f32, tag="pv", name="pv", bufs=1,
                               padded_shape=[128, 512])
                pts = {}
                for j in range(NPAIR + LAG):
                    if j < NPAIR:
                        st = psum.tile([128, 1024], f32, tag="big", name="st",
                                       bufs=3)
                        for h in range(2):
                            mc = 2 * j + h
                            hp = slice(64 * h, 64 * h + 64)
                            nc.tensor.matmul(
                                st[:, h * 512:(h + 1) * 512],
                                ph[hp, mc * 128:(mc + 1) * 128],
                                th[hp, ns],
                                start=True, stop=True,
                                tile_position=(64 * h, 0))
                        ptt = sb.tile([128, 1024], bf16, tag="pt", name="pt",
                                      bufs=LAG + 3)
                        nc.scalar.activation(ptt[:], st[:], AFT.Exp)
                        pts[j] = ptt
                    if nb >= 1 and 1 <= j <= 4:
                        norm_step(nb - 1, *sa_norm[nb - 1], j - 1)
                    if j == 5 and nb >= 1:
                        norm_dma(nb - 1, sa_norm.pop(nb - 1)[1], sc1, 0)
                    if j >= LAG:
                        jj = j - LAG
                        ptt = pts.pop(jj)
                        for h in range(2):
                            mc = 2 * jj + h
                            nc.tensor.matmul(
                                pv[:], gto[:, mc * 65:(mc + 1) * 65],
                                ptt[:, h * 512:(h + 1) * 512],
                                start=(mc == 0), stop=(mc == NM - 1))
                at = sb.tile([65, 512], f32, tag="at", name="at", bufs=2)
                nc.vector.tensor_copy(at[:], pv[:])
                tbt = sb.tile([128, 256], bf16, tag="tb", name="tb", bufs=2)
                sa_norm[nb] = (at, tbt)
            for k in range(4):
                norm_step(NB - 1, *sa_norm[NB - 1], k,
                          tag=("tt" if k % 2 == 0 else "pv"))
            norm_dma(NB - 1, sa_norm.pop(NB - 1)[1], sc1, 0)

            # ------- MoCA: s2 = I@s2f + M2b@lat, 1-lag + spread normalize ----
            moca = {}
            mo_norm = {}
            for nb in range(NB + 2):
                if nb < NB:
                    ns = slice(nb * 512, (nb + 1) * 512)
                    s2 = psum.tile([128, 1024], f32, tag="big", name="s2",
                                   bufs=3)
                    for pc in range(2):
                        nc.tensor.matmul(
                            s2[:, pc * 512:(pc + 1) * 512],
                            id128b[:],
                            s2f[pc][:, ns],
                            start=True, stop=False)
                        nc.tensor.matmul(
                            s2[:, pc * 512:(pc + 1) * 512],
                            m2bb[:, pc * 128:(pc + 1) * 128],
                            latcat[0:64, ns],
                            start=False, stop=True)
                    p2t = sb.tile([128, 1024], bf16, tag="p2t", name="p2t",
                                  bufs=3)
                    nc.scalar.activation(p2t[:], s2[:], AFT.Exp)
                    moca[nb] = p2t
                if nb >= 2:
                    nbll = nb - 2
                    at2, tb2 = mo_norm.pop(nbll)
                    alt = nb >= NB  # trailing norms: pipeline via 2 psum tags
                    for k in range(4):
                        norm_step(nbll, at2, tb2, k,
                                  tag=("pv" if (alt and k % 2 == 1) else "tt"))
                    norm_dma(nbll, tb2, sc2, 64)
                if 1 <= nb <= NB:
                    nbl = nb - 1
                    p2t = moca.pop(nbl)
                    pv2 = psum.tile([65, 512], f32, tag="pv", name="pv2",
                                    bufs=1, padded_shape=[128, 512])
                    for pc in range(2):
                        nc.tensor.matmul(pv2[:], p2w[:, pc * 65:(pc + 1) * 65],
                                         p2t[:, pc * 512:(pc + 1) * 512],
                                         start=(pc == 0), stop=(pc == 1))
                    at2 = sb.tile([65, 512], f32, tag="at", name="at2", bufs=2)
                    nc.vector.tensor_copy(at2[:], pv2[:])
                    tb2 = sb.tile([128, 256], bf16, tag="tb", name="tb2",
                                  bufs=2)
                    mo_norm[nbl] = (at2, tb2)

            # ---------------- tail: out = [wosa|womo]@[lat;lat2] + fm ----
            for nb in range(NB):
                ns = slice(nb * 512, (nb + 1) * 512)
                for g2 in range(2):
                    oc = psum.tile([128, 1024], f32, tag="big", name="oc",
                                   bufs=3)
                    ob = sb.tile([128, 1024], f32, tag="ob", name="ob", bufs=3)
                    for h in range(2):
                        cc = g2 * 2 + h
                        nc.tensor.matmul(oc[:, h * 512:(h + 1) * 512],
                                         wcatb[:, cc * 128:(cc + 1) * 128],
                                         latcat[:, ns],
                                         start=True, stop=True)
                        nc.vector.tensor_add(ob[:, h * 512:(h + 1) * 512],
                                             oc[:, h * 512:(h + 1) * 512],
                                             fmf[cc][nb][:])
                    ov = out_d[g2 * 256:(g2 + 1) * 256, ns].rearrange(
                        "(u p) c -> p u c", u=2)
                    ob_view = ob[:].rearrange("p (u c) -> p u c", u=2)
                    nc.sync.dma_start(ov, ob_view)

    nc.compile()
    return nc


def _get_runner(reps=1):
    """Build the Bass program once and return a cached jitted SPMD callable."""
    key = ("runner", reps)
    if key in _STATE:
        return _STATE[key]

    import jax
    import numpy as np
    from jax.experimental.shard_map import shard_map
    from jax.sharding import Mesh, PartitionSpec
    import concourse.mybir as mybir
    from concourse import bass2jax

    nc = _build_program(reps=reps)
    bass2jax.install_neuronx_cc_hook()

    partition_name = (nc.partition_id_tensor.name
                      if nc.partition_id_tensor else None)
    in_names, out_names, out_avals, zero_shapes = [], [], [], []
    for alloc in nc.m.functions[0].allocations:
        if not isinstance(alloc, mybir.MemoryLocationSet):
            continue
        name = alloc.memorylocations[0].name
        if alloc.kind == "ExternalInput":
            if name != partition_name:
                in_names.append(name)
        elif alloc.kind == "ExternalOutput":
            out_names.append(name)
            shape = tuple(alloc.tensor_shape)
            dtype = mybir.dt.np(alloc.dtype)
            out_avals.append(jax.core.ShapedArray(shape, dtype))
            zero_shapes.append((shape, dtype))
    n_params = len(in_names)
    all_in_names = list(in_names) + list(out_names)
    if partition_name is not None:
        all_in_names.append(partition_name)

    def _body(*args):
        operands = list(args)
        if partition_name is not None:
            operands.append(bass2jax.partition_id_tensor())
        outs = bass2jax._bass_exec_p.bind(
            *operands,
            out_avals=tuple(out_avals),
            in_names=tuple(all_in_names),
            out_names=tuple(out_names),
            lowering_input_output_aliases=(),
            sim_require_finite=True,
            sim_require_nnan=True,
            nc=nc,
        )
        return tuple(outs)

    devices = jax.devices()[:N_CORES]
    mesh = Mesh(np.asarray(devices), ("core",))
    n_outs = len(out_names)
    donate = tuple(range(n_params, n_params + n_outs))
    sharded = jax.jit(
        shard_map(_body, mesh=mesh,
                  in_specs=(PartitionSpec("core"),) * (n_params + n_outs),
                  out_specs=(PartitionSpec("core"),) * n_outs,
                  check_rep=False),
        donate_argnums=donate, keep_unused=True)

    runner = {
        "nc": nc, "sharded": sharded, "in_names": in_names,
        "out_names": out_names, "zero_shapes": zero_shapes,
        "n_params": n_params,
    }
    _STATE[key] = runner
    return runner


def _prep_in_maps(feature_map, concepts, w_theta, w_phi, w_g, w_o,
                  gamma_sa, gamma_moca):
    import ml_dtypes  # noqa: F401  (bf16 host casts if ever needed)

    feature_map = np.asarray(feature_map, dtype=np.float32)
    concepts = np.asarray(concepts, dtype=np.float32)
    w_theta = np.asarray(w_theta, dtype=np.float32)
    w_phi = np.asarray(w_phi, dtype=np.float32)
    w_g = np.asarray(w_g, dtype=np.float32)
    w_o = np.asarray(w_o, dtype=np.float32)
    gamma_sa = np.float32(gamma_sa)
    gamma_moca = np.float32(gamma_moca)

    gain = np.float32(1.0 / np.sqrt(C))
    gain_o = np.float32(1.0 / np.sqrt(L))

    wth_t = w_theta.T * gain                                        # [C, L]
    wph_t = w_phi.T * gain
    wthdup = np.ascontiguousarray(np.concatenate([wth_t, wth_t], axis=1))
    wphdup = np.ascontiguousarray(np.concatenate([wph_t, wph_t], axis=1))
    wg_t = np.ascontiguousarray(w_g.T * gain)                       # [C, L]
    m2 = concepts @ (w_theta * gain)            # [P, C]
    m2t = np.ascontiguousarray(m2.T)            # [C, P]
    m2b = (gamma_sa * gain_o) * (m2 @ w_o)      # [P, L]
    m2bt = np.ascontiguousarray(m2b.T)          # [L, P]
    wosa = w_o.T * (gain_o * gamma_sa)          # [L, C]
    womo = w_o.T * (gain_o * gamma_moca)        # [L, C]
    wcat = np.ascontiguousarray(np.concatenate([wosa, womo], axis=0))
    phiT = np.ascontiguousarray(concepts)                           # [P, L]
    fm_flat = feature_map.reshape(B, C, HW)

    in_maps = []
    for b in range(N_CORES):
        in_maps.append({
            "fm": np.ascontiguousarray(fm_flat[b]),
            "wthdup": wthdup, "wphdup": wphdup, "wg": wg_t,
            "m2t": m2t, "m2bt": m2bt, "wcat": wcat, "phiT": phiT,
        })
    return in_maps


def _run(in_maps):
    r = _get_runner()
    n_params = r["n_params"]
    concat_in = [
        np.concatenate([np.asarray(in_maps[c][name])
                        for c in range(N_CORES)], axis=0)
        for name in r["in_names"]
    ]
    concat_zeros = [np.zeros((N_CORES * s[0], *s[1:]), d)
                    for (s, d) in r["zero_shapes"]]
    out_arrs = r["sharded"](*concat_in, *concat_zeros)
    per_core = []
    for c in range(N_CORES):
        per_core.append({
            name: np.asarray(out_arrs[i]).reshape(
                N_CORES, *r["zero_shapes"][i][0])[c]
            for i, name in enumerate(r["out_names"])
        })
    return per_core


def kernel(feature_map, concepts, w_theta, w_phi, w_g, w_o,
           gamma_sa, gamma_moca):
    in_maps = _prep_in_maps(feature_map, concepts, w_theta, w_phi, w_g, w_o,
                            gamma_sa, gamma_moca)
    per_core = _run(in_maps)
    out = np.stack([per_core[b]["out"].reshape(C, H, W)
                    for b in range(B)], axis=0)
    return out.astype(np.float32)
